# revision 36
# baseline (speedup 1.0000x reference)
"""Expert-parallel MoE feed-forward (top-2 routing) on 8 TRN2 NeuronCores.

Strategy: two expert *half-loads* per core.  The per-expert token counts are
imbalanced (932..984 for this input), so instead of one expert per core
(every core then pads to the max count), each expert's token list is split
into two halves, and every core processes two halves: one "big" expert half
in its chunk-0 column slot and one "small" expert half in its chunk-1 slot.
Static chunk sizes (c1, c2) = (ceil(max_big/2), ceil(max_small/2)) make all
cores identical programs while cutting the per-core column count from
max(n_e) to c1+c2 (984 -> 970 here, ~1.5% less PE work).

Each chunk runs a dense FFN out = (silu(x @ Wg^T) * (x @ Wu^T)) @ Wd^T with
its own weight set, in bf16 (fp32 PSUM accumulation), from SBUF.  The down
weights reuse the gate/up weight buffers (same tile tag -> same SBUF slot;
the Tile framework inserts the WAR dependencies), so both chunks' six weight
matrices fit.  Host scatters per-core outputs back into the (T, A, D)
result.
"""

import math
import sys
import types

import numpy as np
import ml_dtypes

T, D, H, E, A = 4096, 1024, 2048, 8, 2
N_CORES = 8
BF16 = ml_dtypes.bfloat16

# Filled by kernel() with the BassKernelResults of the last device run so an
# external harness (test.py) can read exec_time_ns when tracing is on.
LAST_RESULT = None

_SHIMS_DONE = False


def _install_shims():
    """Environment fixes for running Bass/Tile SPMD kernels under axon."""
    global _SHIMS_DONE
    if _SHIMS_DONE:
        return
    _SHIMS_DONE = True

    # 1. NTFF profile hook (lets trace=True / BASS_TRACE=1 report exec_time_ns).
    if "antenv.axon_hooks" not in sys.modules:
        try:
            import antenv.axon_hooks  # noqa: F401  (real module present)
        except ImportError:
            _hook = None
            try:
                import trn_agent_boot.trn_boot as tb

                _hook = tb._ntff_profile_via_ctypes("/opt/axon/libaxon_pjrt.so")
            except Exception:
                _hook = None
            mod = types.ModuleType("antenv.axon_hooks")
            mod.get_axon_ntff_profile_hook = lambda: _hook
            sys.modules["antenv.axon_hooks"] = mod

    # 2. No artifact upload from a zero-egress container.
    from concourse import bass_utils

    bass_utils.upload_artifacts = lambda tmpdir: f"local:{tmpdir}"

    # 3. Slim tile-exit: keep the drain (its sem waits are what guarantee the
    # final output DMAs have landed before the engines halt) but drop the
    # all-engine barriers and the semaphore RANGE_CLEAR.  The NEFF epilogue
    # opens with its own all-engine barrier and then resets the entire
    # 256-semaphore space, so the tile-side clear+barriers only add ~1 us of
    # teardown inside the measured window.  Also split the drain's sem waits
    # onto nops (this walrus build allows one sync-wait command per
    # instruction).
    import concourse.tile as tile
    from concourse import mybir
    from concourse.vector_clock import ScopedClock

    if getattr(tile.TileContext._drain_and_barrier, "_is_patched", False):
        return

    def _patched_drain_and_barrier(self, tick_clock, wait_clock):
        nc = self.nc
        drain_inst = nc.sync.drain()
        wait_clock.add_sem_waits(
            drain_inst.ins, ScopedClock({None: tick_clock.global_clock})
        )
        ow = drain_inst.ins.sync_info.on_wait if drain_inst.ins.sync_info else None
        maxw = 1
        if ow and len(ow) > maxw:
            extra = list(ow[maxw:])
            del ow[maxw:]
            for i in range(0, len(extra), maxw):
                nop = nc.sync.nop(hint="drain_split", nofuse=True)
                if nop.ins.sync_info is None:
                    nop.ins.sync_info = mybir.SyncInfo(on_wait=[], on_update=[])
                for w in extra[i : i + maxw]:
                    nop.ins.sync_info.on_wait.append(w)
        assert self.sems is not None
        popped = nc._tile_sem_poison_stack.pop()
        assert popped is self._sem_poison
        # Python-side bookkeeping of clear_and_free_semaphores, with no
        # emitted instructions (the NEFF epilogue resets every semaphore).
        sems = list(self.sems.allocated().values())
        sem_nums = [s.num if hasattr(s, "num") else s for s in sems]
        nc._state.prepend_free_semaphores(sem_nums)
        for poison_set in nc._tile_sem_poison_stack:
            poison_set.update(sem_nums)

    _patched_drain_and_barrier._is_patched = True
    tile.TileContext._drain_and_barrier = _patched_drain_and_barrier


def _split_multi_waits(nc):
    """This walrus build allows one sync-wait command per instruction.

    Tile's sem assignment can attach several; move the extras onto nofuse
    NoOps inserted just before the instruction on the same engine (engines
    execute a block's instructions in order, so semantics are unchanged).
    """
    import bass_rust
    from concourse import mybir

    ctr = 0
    for f in nc.m.functions:
        for bb in f.blocks:
            new = []
            changed = False
            for inst in bb.instructions:
                si = inst.sync_info
                ow = si.on_wait if si else None
                if ow is not None and len(ow) > 1:
                    extra = list(ow[:-1])
                    del ow[:-1]
                    for w in extra:
                        ctr += 1
                        nop = bass_rust.InstNoOp()
                        nop.name = f"I-wsplit-{ctr}"
                        nop.engine = inst.engine
                        nop.sync_info = mybir.SyncInfo(on_wait=[w], on_update=[])
                        nop.bass_nofuse = True
                        new.append(nop)
                    changed = True
                new.append(inst)
            if changed:
                bb.instructions = new


_NC_CACHE = {}


def _build_nc(c1, c2):
    key = (c1, c2)
    if key in _NC_CACHE:
        return _NC_CACHE[key]
    import concourse.bass as bass
    import concourse.tile as tile
    from concourse import mybir

    f32 = mybir.dt.float32
    bf16 = mybir.dt.bfloat16
    KD = D // 128  # 8  k-tiles over the model dim
    KH = H // 128  # 16 k-tiles over the hidden dim
    cap = c1 + c2
    chunks = [(0, c1, 0), (c1, c2, 1)]  # (col offset, width, weight set)
    cmax = max(c1, c2)

    nc = bass.Bass()
    # Host pre-packs every input into the per-partition-contiguous layout the
    # SBUF tiles use ([p, ki*W + c] = src[ki*128 + p, c]).  A DMA then moves
    # 128 DRAM rows whose length grows with the k-range — and queue bandwidth
    # is descriptor-row-rate-bound (~15 ns/row regardless of size), so fat
    # rows are the difference between 65 GB/s and wire speed per queue.
    xT = nc.dram_tensor("xT", [128, KD * cap], bf16, kind="ExternalInput")
    wgT = [
        nc.dram_tensor(f"wgT{s}", [128, KD * H], bf16, kind="ExternalInput")
        for s in range(2)
    ]
    wuT = [
        nc.dram_tensor(f"wuT{s}", [128, KD * H], bf16, kind="ExternalInput")
        for s in range(2)
    ]
    wdT = [
        nc.dram_tensor(f"wdT{s}", [128, KH * D], bf16, kind="ExternalInput")
        for s in range(2)
    ]
    out = nc.dram_tensor("out", [D, cap], bf16, kind="ExternalOutput")

    GCOL = 1024  # gate/up group-A column frontier (hi 0..7 of each k-tile)

    with tile.TileContext(nc) as tc:
        with (
            tc.tile_pool(name="wpool", bufs=1) as wpool,
            tc.tile_pool(name="hpool", bufs=2) as hpool,
            tc.tile_pool(name="opool", bufs=4) as opool,
            tc.tile_pool(name="psum", bufs=2, space="PSUM") as psum,
        ):
            x_all = wpool.tile([128, KD * cap], bf16, tag="x", name="x_all")
            wg_all = [
                wpool.tile([128, KD * H], bf16, tag=f"wg{s}", name=f"wg_all{s}")
                for s in range(2)
            ]
            wu_all = [
                wpool.tile([128, KD * H], bf16, tag=f"wu{s}", name=f"wu_all{s}")
                for s in range(2)
            ]
            zw = wpool.tile([128, 256], bf16, tag="zw", name="zw_sb")
            zs = wpool.tile([128, 16], bf16, tag="zs", name="zs_sb")
            nc.vector.memset(zw[:], 0.0)
            nc.vector.memset(zs[:, :8], 0.0)

            # PE warmup on the "po" PSUM banks, whose first real use (the
            # down phase) is far away: the first real matmul takes no
            # dependency on these, so they purely absorb the 0.65/1.2 GHz
            # DVFS ramp during the DMA-wait head.
            warm = psum.tile([128, 512], f32, tag="pp7", bufs=1, name="warm")
            for _ in range(12):
                nc.tensor.matmul(
                    warm[:, :256], zw[:, :128], zw[:], start=True, stop=True
                )

            def dma_x(eng, a, b):
                eng.dma_start(x_all[:, a * cap : b * cap], xT[:, a * cap : b * cap])

            def dma_w(eng, w_all, wsrc, ki, c0, cc, W=H):
                sl = slice(ki * W + c0, ki * W + cc)
                eng.dma_start(w_all[:, sl], wsrc[:, sl])

            # --- input DMA plan ---
            # Queue behavior (measured): GpSimd's SWDGE queue sustains ~200
            # GB/s; SP/ACT HWDGE ~100-145 each, and the ACT queue throttles
            # hard once the engine runs silus; per-core HBM tops out near
            # ~380 GB/s aggregate; each queue has 1.3-2.3 us cold-start.
            # The gate phase consumes (x[ki], wgA[ki]) every ~1.3-1.4 us in k
            # order, so the critical stream spreads over all three queues in
            # consumption order; bulky late-deadline tensors follow strictly
            # after it; ACT gets nothing with a late deadline.
            wg0, wu0 = wg_all[0], wu_all[0]
            # SP (q1, earliest cold-start) carries wgA[k0] — the piece that
            # actually gates the first real matmul; ACT (latest cold-start)
            # gets x[k0], whose slack is similar.  Whole tiles, not splits:
            # a later but uninterrupted PE start beats an earlier start with
            # stalls (the DVFS ramp rewards sustained activity).
            dma_w(nc.sync, wg0, wgT[0], 0, 0, GCOL)
            dma_w(nc.sync, wg0, wgT[0], 2, 0, GCOL)
            dma_x(nc.sync, 4, 5)
            dma_w(nc.sync, wg0, wgT[0], 6, 0, GCOL)
            # ACT (q10): only early pieces.
            dma_x(nc.scalar, 0, 1)
            dma_x(nc.scalar, 2, 3)
            dma_w(nc.scalar, wg0, wgT[0], 3, 0, GCOL)
            dma_x(nc.scalar, 6, 7)
            # ACT loads its activation table lazily on the first ACTIVATE
            # (~1.3 us); trigger it on dummy data now so group 1's silus —
            # whose PSUM-bank releases gate group 2's matmuls — start sooner.
            nc.scalar.activation(
                zs[:, 8:16], zs[:, :8], mybir.ActivationFunctionType.Silu
            )
            # GpSimd (q0): the rest of the critical stream, k-ordered.
            dma_x(nc.gpsimd, 1, 2)
            dma_w(nc.gpsimd, wg0, wgT[0], 1, 0, GCOL)
            dma_x(nc.gpsimd, 3, 4)
            dma_w(nc.gpsimd, wg0, wgT[0], 4, 0, GCOL)
            dma_x(nc.gpsimd, 5, 6)
            dma_w(nc.gpsimd, wg0, wgT[0], 5, 0, GCOL)
            dma_x(nc.gpsimd, 7, 8)
            dma_w(nc.gpsimd, wg0, wgT[0], 7, 0, GCOL)
            # Gate-0 weights groups B+C (cols GCOL:H), k-ordered, SP/GpSimd.
            dma_w(nc.sync, wg0, wgT[0], 0, GCOL, H)
            dma_w(nc.gpsimd, wg0, wgT[0], 1, GCOL, H)
            dma_w(nc.gpsimd, wg0, wgT[0], 2, GCOL, H)
            dma_w(nc.sync, wg0, wgT[0], 3, GCOL, H)
            dma_w(nc.gpsimd, wg0, wgT[0], 4, GCOL, H)
            dma_w(nc.sync, wg0, wgT[0], 5, GCOL, H)
            dma_w(nc.gpsimd, wg0, wgT[0], 6, GCOL, H)
            dma_w(nc.gpsimd, wg0, wgT[0], 7, GCOL, H)
            # Up-0 weights (consumed k-ordered once chunk 0's gate phase
            # ends): 2-k-tile fat rows, k-pairs alternating SP/GpSimd so
            # arrival order matches consumption order under shared HBM.
            dma_up0 = [
                (nc.sync, 0),
                (nc.gpsimd, 1),
                (nc.sync, 2),
                (nc.gpsimd, 3),
            ]
            for eng, j in dma_up0:
                eng.dma_start(
                    wu0[:, 2 * j * H : 2 * (j + 1) * H],
                    wuT[0][:, 2 * j * H : 2 * (j + 1) * H],
                )
            # Chunk-1 gate weights: per-k-tile full rows, k-ordered, so their
            # arrival paces gate(c1)'s consumption instead of front-loading
            # HBM bandwidth that tighter-deadline tensors need.
            for ki in range(KD):
                nc.gpsimd.dma_start(
                    wg_all[1][:, ki * H : (ki + 1) * H],
                    wgT[1][:, ki * H : (ki + 1) * H],
                )
            nc.sync.dma_start(wu_all[1][:, 0 : 4 * H], wuT[1][:, 0 : 4 * H])
            nc.gpsimd.dma_start(wu_all[1][:, 4 * H : 8 * H], wuT[1][:, 4 * H : 8 * H])
            # NOTE on the down weights (emitted below, after gate_up(c1)):
            # they reuse SBUF slots, so their DMA instructions carry WAR
            # waits that PARK the issuing engine until the reusee's readers
            # finish; a parked engine stops feeding its descriptor ring and
            # starves its whole queue.  So: wd0 reuses WU0's slot (WAR
            # clears ~66 us, when GpSimd's ring has fully drained) and wd1
            # reuses wg0's (WAR ~42, already satisfied) — both are GpSimd's
            # last items, so nothing can starve behind them, and the late
            # transfer window keeps them from stealing HBM bandwidth from
            # tighter-deadline weights.

            def gate_up(c0, cn, s):
                # Phase 1: all gate matmuls; silu lands bf16 directly in h.
                # Phase 2: all up matmuls; h *= pu in place on the DVE.
                # Phasing delays the first need for wu by a whole gate phase.
                # Within a phase, k is the OUTER loop over groups of 6 h-tiles
                # accumulating in 6 PSUM banks: weight consumption order then
                # matches the k-major DMA arrival order, so the PE never
                # outruns the transfer frontier during the startup ramp.
                h_sb = hpool.tile([128, KH * cmax], bf16, tag="h", name="h_sb")
                csl = slice(c0, c0 + cn)
                wg_s, wu_s = wg_all[s], wu_all[s]

                def phase(w_all, writer):
                    for g0 in range(0, KH, 8):
                        his = range(g0, min(g0 + 8, KH))
                        pp = [
                            psum.tile(
                                [128, 512], f32, tag=f"pp{j}", bufs=1, name=f"pp{j}"
                            )
                            for j in range(len(his))
                        ]
                        for ki in range(KD):
                            for j, hi in enumerate(his):
                                nc.tensor.matmul(
                                    pp[j][:, :cn],
                                    w_all[:, ki * H + 128 * hi : ki * H + 128 * (hi + 1)],
                                    x_all[:, ki * cap + c0 : ki * cap + c0 + cn],
                                    start=(ki == 0),
                                    stop=(ki == KD - 1),
                                )
                        for j, hi in enumerate(his):
                            writer(hi, pp[j])

                def gate_writer(hi, pp):
                    nc.scalar.activation(
                        h_sb[:, cmax * hi : cmax * hi + cn],
                        pp[:, :cn],
                        mybir.ActivationFunctionType.Silu,
                    )

                def up_writer(hi, pp):
                    hslc = slice(cmax * hi, cmax * hi + cn)
                    nc.vector.tensor_mul(h_sb[:, hslc], h_sb[:, hslc], pp[:, :cn])

                phase(wg_s, gate_writer)
                phase(wu_s, up_writer)
                return h_sb

            down_ctr = [0]

            def down(h_sb, c0, cn, wd_sb, last_chunk):
                for di in range(KD):
                    # The very last d-tile runs as two column groups so its
                    # cast+store pipelines against its own matmuls instead of
                    # sitting fully exposed after the final one.
                    if last_chunk and di == KD - 1:
                        cgroups = [(0, cn // 2), (cn // 2, cn)]
                    else:
                        cgroups = [(0, cn)]
                    for g0, g1 in cgroups:
                        gw = g1 - g0
                        po = psum.tile(
                            [128, 512],
                            f32,
                            tag=f"pp{down_ctr[0] % 2}",
                            bufs=1,
                            name="po",
                        )
                        down_ctr[0] += 1
                        for hk in range(KH):
                            nc.tensor.matmul(
                                po[:, :gw],
                                wd_sb[:, hk * D + 128 * di : hk * D + 128 * (di + 1)],
                                h_sb[:, cmax * hk + g0 : cmax * hk + g1],
                                start=(hk == 0),
                                stop=(hk == KH - 1),
                            )
                        o = opool.tile([128, 512], bf16, tag="o", name="o")
                        nc.vector.tensor_copy(o[:, :gw], po[:, :gw])
                        # Store queue time is DRAM-row-rate-bound and these
                        # rows are thin, so split by PARTITION to halve the
                        # rows each queue processes.
                        d0 = 128 * di
                        csl2 = slice(c0 + g0, c0 + g1)
                        nc.sync.dma_start(out[d0 : d0 + 64, csl2], o[0:64, :gw])
                        nc.scalar.dma_start(
                            out[d0 + 64 : d0 + 128, csl2], o[64:128, :gw]
                        )

            # Software-pipelined emission: down(c) goes after gate_up(c+1) so
            # the PE can run chunk c+1's gate matmuls while the DVE finishes
            # chunk c's h tiles (h is double-buffered).
            h0 = gate_up(0, c1, 0)
            h1 = gate_up(c1, c2, 1)
            # Down weights reuse chunk-0 weight slots (same tag -> same SBUF
            # slot; Tile inserts the WAR dependencies) — see NOTE above.
            wd_sb = [
                wpool.tile([128, KH * D], bf16, tag="wu0", name="wd_all0"),
                wpool.tile([128, KH * D], bf16, tag="wg0", name="wd_all1"),
            ]
            nc.gpsimd.dma_start(wd_sb[0][:, 0 : 8 * D], wdT[0][:, 0 : 8 * D])
            nc.gpsimd.dma_start(wd_sb[0][:, 8 * D : 16 * D], wdT[0][:, 8 * D : 16 * D])
            nc.gpsimd.dma_start(wd_sb[1][:, 0 : 8 * D], wdT[1][:, 0 : 8 * D])
            nc.gpsimd.dma_start(wd_sb[1][:, 8 * D : 16 * D], wdT[1][:, 8 * D : 16 * D])
            down(h0, 0, c1, wd_sb[0], False)
            down(h1, c1, c2, wd_sb[1], True)
    _split_multi_waits(nc)
    _NC_CACHE[key] = nc
    return nc


def kernel(x, expert_indices, w_gate, w_up, w_down):
    global LAST_RESULT
    _install_shims()
    from concourse import bass_utils

    x = np.asarray(x)
    ei = np.asarray(expert_indices).astype(np.int64)
    w_gate = np.asarray(w_gate)
    w_up = np.asarray(w_up)
    w_down = np.asarray(w_down)

    flat = ei.reshape(-1)  # pair p = t*A + a  ->  expert id
    # Dedup: a (token, slot) pair whose expert already appears in an earlier
    # slot of the same token produces an identical output row — compute the
    # first occurrence only and copy the result to the duplicates afterward.
    keep = np.ones(T * A, dtype=bool)
    for a in range(1, A):
        dup_any = np.zeros(T, dtype=bool)
        for b in range(a):
            dup_any |= ei[:, a] == ei[:, b]
        keep[a::A] = ~dup_any[:T]
    kept = np.nonzero(keep)[0]
    flat_kept = flat[kept]
    counts = np.bincount(flat_kept, minlength=E)
    order = np.argsort(flat_kept, kind="stable")
    starts = np.zeros(E + 1, dtype=np.int64)
    np.cumsum(counts, out=starts[1:])

    # Two half-loads per core: sort experts by count; the E/2 biggest go to
    # the cores' chunk-0 slots (split in half across core pairs), the E/2
    # smallest to the chunk-1 slots.
    assert E == N_CORES
    by_size = np.argsort(-counts, kind="stable")
    big, small = by_size[: E // 2], by_size[E // 2 :]
    c1 = max(128, int(math.ceil(counts[big[0]] / 2)))
    c2 = max(128, int(math.ceil(counts[small[0]] / 2)))
    assert c1 <= 512 and c2 <= 512, (c1, c2)
    cap = c1 + c2

    KD = D // 128
    KH = H // 128

    def pack(a, k, w):
        # [k*128, w] -> [128, k*w] with [p, ki*w + c] = a[ki*128 + p, c]
        return np.ascontiguousarray(
            a.reshape(k, 128, w).transpose(1, 0, 2).reshape(128, k * w)
        )

    wpk = {}
    for e in range(E):
        wpk[e] = (
            pack(w_gate[e].T.astype(BF16), KD, H),
            pack(w_up[e].T.astype(BF16), KD, H),
            pack(w_down[e].T.astype(BF16), KH, D),
        )

    # slot assignment: core 2i/2i+1 chunk0 <- halves of big[i]; chunk1 <-
    # halves of small[i].
    slot_expert = np.zeros((N_CORES, 2), dtype=np.int64)
    slot_idx = [[None, None] for _ in range(N_CORES)]
    for i in range(E // 2):
        for s, exp_list in ((0, big), (1, small)):
            e = exp_list[i]
            idx = kept[order[starts[e] : starts[e + 1]]]
            half = (len(idx) + 1) // 2
            for j, piece in enumerate((idx[:half], idx[half:])):
                core = 2 * i + j
                slot_expert[core, s] = e
                slot_idx[core][s] = piece

    in_maps = []
    for core in range(N_CORES):
        xeT = np.zeros((D, cap), dtype=BF16)
        for s, coff, cw in ((0, 0, c1), (1, c1, c2)):
            idx = slot_idx[core][s]
            tok = idx // A
            xeT[:, coff : coff + len(idx)] = x[tok].T.astype(BF16)
        e0, e1 = slot_expert[core]
        in_maps.append(
            {
                "xT": pack(xeT, KD, cap),
                "wgT0": wpk[e0][0],
                "wuT0": wpk[e0][1],
                "wdT0": wpk[e0][2],
                "wgT1": wpk[e1][0],
                "wuT1": wpk[e1][1],
                "wdT1": wpk[e1][2],
            }
        )

    nc = _build_nc(c1, c2)
    res = bass_utils.run_bass_kernel_spmd(nc, in_maps, core_ids=list(range(N_CORES)))
    LAST_RESULT = res

    out = np.zeros((T * A, D), dtype=np.float32)
    for core in range(N_CORES):
        oT = np.asarray(res.results[core]["out"])  # [D, cap] bf16
        for s, coff in ((0, 0), (1, c1)):
            idx = slot_idx[core][s]
            out[idx] = oT[:, coff : coff + len(idx)].T.astype(np.float32)
    out = out.reshape(T, A, D)
    for a in range(1, A):  # fill duplicate slots from their first occurrence
        for b in range(a):
            m = ei[:, a] == ei[:, b]
            if b > 0:
                for c in range(b):
                    m &= ei[:, b] != ei[:, c]  # b is itself the first occurrence
            out[m, a] = out[m, b]
    return out


# revision 37
# speedup vs baseline: 1.0288x; 1.0288x over previous
"""Expert-parallel MoE feed-forward (top-2 routing) on 8 TRN2 NeuronCores.

Strategy: two expert *half-loads* per core.  The per-expert token counts are
imbalanced (932..984 for this input), so instead of one expert per core
(every core then pads to the max count), each expert's token list is split
into two halves, and every core processes two halves: one "big" expert half
in its chunk-0 column slot and one "small" expert half in its chunk-1 slot.
Static chunk sizes (c1, c2) = (ceil(max_big/2), ceil(max_small/2)) make all
cores identical programs while cutting the per-core column count from
max(n_e) to c1+c2 (984 -> 970 here, ~1.5% less PE work).

Each chunk runs a dense FFN out = (silu(x @ Wg^T) * (x @ Wu^T)) @ Wd^T with
its own weight set, in bf16 (fp32 PSUM accumulation), from SBUF.  The down
weights reuse the gate/up weight buffers (same tile tag -> same SBUF slot;
the Tile framework inserts the WAR dependencies), so both chunks' six weight
matrices fit.  Host scatters per-core outputs back into the (T, A, D)
result.
"""

import math
import sys
import types

import numpy as np
import ml_dtypes

T, D, H, E, A = 4096, 1024, 2048, 8, 2
N_CORES = 8
BF16 = ml_dtypes.bfloat16

# Filled by kernel() with the BassKernelResults of the last device run so an
# external harness (test.py) can read exec_time_ns when tracing is on.
LAST_RESULT = None

_SHIMS_DONE = False


def _install_shims():
    """Environment fixes for running Bass/Tile SPMD kernels under axon."""
    global _SHIMS_DONE
    if _SHIMS_DONE:
        return
    _SHIMS_DONE = True

    # 1. NTFF profile hook (lets trace=True / BASS_TRACE=1 report exec_time_ns).
    if "antenv.axon_hooks" not in sys.modules:
        try:
            import antenv.axon_hooks  # noqa: F401  (real module present)
        except ImportError:
            _hook = None
            try:
                import trn_agent_boot.trn_boot as tb

                _hook = tb._ntff_profile_via_ctypes("/opt/axon/libaxon_pjrt.so")
            except Exception:
                _hook = None
            mod = types.ModuleType("antenv.axon_hooks")
            mod.get_axon_ntff_profile_hook = lambda: _hook
            sys.modules["antenv.axon_hooks"] = mod

    # 2. No artifact upload from a zero-egress container.
    from concourse import bass_utils

    bass_utils.upload_artifacts = lambda tmpdir: f"local:{tmpdir}"

    # 3. Slim tile-exit: keep the drain (its sem waits are what guarantee the
    # final output DMAs have landed before the engines halt) but drop the
    # all-engine barriers and the semaphore RANGE_CLEAR.  The NEFF epilogue
    # opens with its own all-engine barrier and then resets the entire
    # 256-semaphore space, so the tile-side clear+barriers only add ~1 us of
    # teardown inside the measured window.  Also split the drain's sem waits
    # onto nops (this walrus build allows one sync-wait command per
    # instruction).
    import concourse.tile as tile
    from concourse import mybir
    from concourse.vector_clock import ScopedClock

    if getattr(tile.TileContext._drain_and_barrier, "_is_patched", False):
        return

    def _patched_drain_and_barrier(self, tick_clock, wait_clock):
        nc = self.nc
        drain_inst = nc.sync.drain()
        wait_clock.add_sem_waits(
            drain_inst.ins, ScopedClock({None: tick_clock.global_clock})
        )
        ow = drain_inst.ins.sync_info.on_wait if drain_inst.ins.sync_info else None
        maxw = 1
        if ow and len(ow) > maxw:
            extra = list(ow[maxw:])
            del ow[maxw:]
            for i in range(0, len(extra), maxw):
                nop = nc.sync.nop(hint="drain_split", nofuse=True)
                if nop.ins.sync_info is None:
                    nop.ins.sync_info = mybir.SyncInfo(on_wait=[], on_update=[])
                for w in extra[i : i + maxw]:
                    nop.ins.sync_info.on_wait.append(w)
        assert self.sems is not None
        popped = nc._tile_sem_poison_stack.pop()
        assert popped is self._sem_poison
        # Python-side bookkeeping of clear_and_free_semaphores, with no
        # emitted instructions (the NEFF epilogue resets every semaphore).
        sems = list(self.sems.allocated().values())
        sem_nums = [s.num if hasattr(s, "num") else s for s in sems]
        nc._state.prepend_free_semaphores(sem_nums)
        for poison_set in nc._tile_sem_poison_stack:
            poison_set.update(sem_nums)

    _patched_drain_and_barrier._is_patched = True
    tile.TileContext._drain_and_barrier = _patched_drain_and_barrier


def _split_multi_waits(nc):
    """This walrus build allows one sync-wait command per instruction.

    Tile's sem assignment can attach several; move the extras onto nofuse
    NoOps inserted just before the instruction on the same engine (engines
    execute a block's instructions in order, so semantics are unchanged).
    """
    import bass_rust
    from concourse import mybir

    ctr = 0
    for f in nc.m.functions:
        for bb in f.blocks:
            new = []
            changed = False
            for inst in bb.instructions:
                si = inst.sync_info
                ow = si.on_wait if si else None
                if ow is not None and len(ow) > 1:
                    extra = list(ow[:-1])
                    del ow[:-1]
                    for w in extra:
                        ctr += 1
                        nop = bass_rust.InstNoOp()
                        nop.name = f"I-wsplit-{ctr}"
                        nop.engine = inst.engine
                        nop.sync_info = mybir.SyncInfo(on_wait=[w], on_update=[])
                        nop.bass_nofuse = True
                        new.append(nop)
                    changed = True
                new.append(inst)
            if changed:
                bb.instructions = new


_NC_CACHE = {}


def _build_nc(c1, c2):
    key = (c1, c2)
    if key in _NC_CACHE:
        return _NC_CACHE[key]
    import concourse.bass as bass
    import concourse.tile as tile
    from concourse import mybir

    f32 = mybir.dt.float32
    bf16 = mybir.dt.bfloat16
    KD = D // 128  # 8  k-tiles over the model dim
    KH = H // 128  # 16 k-tiles over the hidden dim
    cap = c1 + c2
    chunks = [(0, c1, 0), (c1, c2, 1)]  # (col offset, width, weight set)
    cmax = max(c1, c2)

    nc = bass.Bass()
    # Host pre-packs every input into the per-partition-contiguous layout the
    # SBUF tiles use ([p, ki*W + c] = src[ki*128 + p, c]).  A DMA then moves
    # 128 DRAM rows whose length grows with the k-range — and queue bandwidth
    # is descriptor-row-rate-bound (~15 ns/row regardless of size), so fat
    # rows are the difference between 65 GB/s and wire speed per queue.
    xT = nc.dram_tensor("xT", [128, KD * cap], bf16, kind="ExternalInput")
    wgT = [
        nc.dram_tensor(f"wgT{s}", [128, KD * H], bf16, kind="ExternalInput")
        for s in range(2)
    ]
    wuT = [
        nc.dram_tensor(f"wuT{s}", [128, KD * H], bf16, kind="ExternalInput")
        for s in range(2)
    ]
    wdT = [
        nc.dram_tensor(f"wdT{s}", [128, KH * D], bf16, kind="ExternalInput")
        for s in range(2)
    ]
    out = nc.dram_tensor("out", [D, cap], bf16, kind="ExternalOutput")

    GCOL = 1024  # gate/up group-A column frontier (hi 0..7 of each k-tile)

    with tile.TileContext(nc) as tc:
        with (
            tc.tile_pool(name="wpool", bufs=1) as wpool,
            tc.tile_pool(name="hpool", bufs=2) as hpool,
            tc.tile_pool(name="opool", bufs=4) as opool,
            tc.tile_pool(name="psum", bufs=2, space="PSUM") as psum,
        ):
            x_all = wpool.tile([128, KD * cap], bf16, tag="x", name="x_all")
            wg_all = [
                wpool.tile([128, KD * H], bf16, tag=f"wg{s}", name=f"wg_all{s}")
                for s in range(2)
            ]
            wu_all = [
                wpool.tile([128, KD * H], bf16, tag=f"wu{s}", name=f"wu_all{s}")
                for s in range(2)
            ]
            zw = wpool.tile([128, 256], bf16, tag="zw", name="zw_sb")
            zs = wpool.tile([128, 16], bf16, tag="zs", name="zs_sb")
            nc.vector.memset(zw[:], 0.0)
            nc.vector.memset(zs[:, :8], 0.0)

            # PE warmup on the "po" PSUM banks, whose first real use (the
            # down phase) is far away: the first real matmul takes no
            # dependency on these, so they purely absorb the 0.65/1.2 GHz
            # DVFS ramp during the DMA-wait head.
            warm = psum.tile([128, 512], f32, tag="pp7", bufs=1, name="warm")
            for _ in range(12):
                nc.tensor.matmul(
                    warm[:, :256], zw[:, :128], zw[:], start=True, stop=True
                )

            def dma_x(eng, a, b):
                eng.dma_start(x_all[:, a * cap : b * cap], xT[:, a * cap : b * cap])

            def dma_w(eng, w_all, wsrc, ki, c0, cc, W=H):
                sl = slice(ki * W + c0, ki * W + cc)
                eng.dma_start(w_all[:, sl], wsrc[:, sl])

            # --- input DMA plan ---
            # Queue behavior (measured): GpSimd's SWDGE queue sustains ~200
            # GB/s; SP/ACT HWDGE ~100-145 each, and the ACT queue throttles
            # hard once the engine runs silus; per-core HBM tops out near
            # ~380 GB/s aggregate; each queue has 1.3-2.3 us cold-start.
            # The gate phase consumes (x[ki], wgA[ki]) every ~1.3-1.4 us in k
            # order, so the critical stream spreads over all three queues in
            # consumption order; bulky late-deadline tensors follow strictly
            # after it; ACT gets nothing with a late deadline.
            wg0, wu0 = wg_all[0], wu_all[0]
            # SP (q1, warms up first): x[k0] whole tile — a later but
            # uninterrupted PE start beats an earlier start with stalls (the
            # DVFS ramp rewards sustained activity).
            dma_x(nc.sync, 0, 1)
            dma_w(nc.sync, wg0, wgT[0], 2, 0, GCOL)
            dma_x(nc.sync, 4, 5)
            dma_w(nc.sync, wg0, wgT[0], 6, 0, GCOL)
            # ACT (q10): only early pieces.
            dma_w(nc.scalar, wg0, wgT[0], 0, 0, GCOL)
            dma_x(nc.scalar, 2, 3)
            dma_w(nc.scalar, wg0, wgT[0], 3, 0, GCOL)
            dma_x(nc.scalar, 6, 7)
            # ACT loads its activation table lazily on the first ACTIVATE
            # (~1.3 us); trigger it on dummy data now so group 1's silus —
            # whose PSUM-bank releases gate group 2's matmuls — start sooner.
            nc.scalar.activation(
                zs[:, 8:16], zs[:, :8], mybir.ActivationFunctionType.Silu
            )
            # GpSimd (q0): the rest of the critical stream, k-ordered.
            dma_x(nc.gpsimd, 1, 2)
            dma_w(nc.gpsimd, wg0, wgT[0], 1, 0, GCOL)
            dma_x(nc.gpsimd, 3, 4)
            dma_w(nc.gpsimd, wg0, wgT[0], 4, 0, GCOL)
            dma_x(nc.gpsimd, 5, 6)
            dma_w(nc.gpsimd, wg0, wgT[0], 5, 0, GCOL)
            dma_x(nc.gpsimd, 7, 8)
            dma_w(nc.gpsimd, wg0, wgT[0], 7, 0, GCOL)
            # Gate-0 weights groups B+C (cols GCOL:H), k-ordered, SP/GpSimd.
            dma_w(nc.sync, wg0, wgT[0], 0, GCOL, H)
            dma_w(nc.gpsimd, wg0, wgT[0], 1, GCOL, H)
            dma_w(nc.gpsimd, wg0, wgT[0], 2, GCOL, H)
            dma_w(nc.sync, wg0, wgT[0], 3, GCOL, H)
            dma_w(nc.gpsimd, wg0, wgT[0], 4, GCOL, H)
            dma_w(nc.sync, wg0, wgT[0], 5, GCOL, H)
            dma_w(nc.gpsimd, wg0, wgT[0], 6, GCOL, H)
            dma_w(nc.gpsimd, wg0, wgT[0], 7, GCOL, H)
            # Up-0 weights (consumed k-ordered once chunk 0's gate phase
            # ends): 2-k-tile fat rows, k-pairs alternating SP/GpSimd so
            # arrival order matches consumption order under shared HBM.
            dma_up0 = [
                (nc.sync, 0),
                (nc.gpsimd, 1),
                (nc.sync, 2),
                (nc.gpsimd, 3),
            ]
            for eng, j in dma_up0:
                eng.dma_start(
                    wu0[:, 2 * j * H : 2 * (j + 1) * H],
                    wuT[0][:, 2 * j * H : 2 * (j + 1) * H],
                )
            # Chunk-1 gate weights: per-k-tile full rows, k-ordered, so their
            # arrival paces gate(c1)'s consumption instead of front-loading
            # HBM bandwidth that tighter-deadline tensors need.
            for ki in range(KD):
                nc.gpsimd.dma_start(
                    wg_all[1][:, ki * H : (ki + 1) * H],
                    wgT[1][:, ki * H : (ki + 1) * H],
                )
            nc.sync.dma_start(wu_all[1][:, 0 : 4 * H], wuT[1][:, 0 : 4 * H])
            nc.gpsimd.dma_start(wu_all[1][:, 4 * H : 8 * H], wuT[1][:, 4 * H : 8 * H])
            # NOTE on the down weights (emitted below, after gate_up(c1)):
            # they reuse SBUF slots, so their DMA instructions carry WAR
            # waits that PARK the issuing engine until the reusee's readers
            # finish; a parked engine stops feeding its descriptor ring and
            # starves its whole queue.  So: wd0 reuses WU0's slot (WAR
            # clears ~66 us, when GpSimd's ring has fully drained) and wd1
            # reuses wg0's (WAR ~42, already satisfied) — both are GpSimd's
            # last items, so nothing can starve behind them, and the late
            # transfer window keeps them from stealing HBM bandwidth from
            # tighter-deadline weights.

            def gate_up(c0, cn, s):
                # Phase 1: all gate matmuls; silu lands bf16 directly in h.
                # Phase 2: all up matmuls; h *= pu in place on the DVE.
                # Phasing delays the first need for wu by a whole gate phase.
                # Within a phase, k is the OUTER loop over groups of 6 h-tiles
                # accumulating in 6 PSUM banks: weight consumption order then
                # matches the k-major DMA arrival order, so the PE never
                # outruns the transfer frontier during the startup ramp.
                h_sb = hpool.tile([128, KH * cmax], bf16, tag="h", name="h_sb")
                csl = slice(c0, c0 + cn)
                wg_s, wu_s = wg_all[s], wu_all[s]

                def phase(w_all, writer):
                    for g0 in range(0, KH, 8):
                        his = range(g0, min(g0 + 8, KH))
                        pp = [
                            psum.tile(
                                [128, 512], f32, tag=f"pp{j}", bufs=1, name=f"pp{j}"
                            )
                            for j in range(len(his))
                        ]
                        for ki in range(KD):
                            for j, hi in enumerate(his):
                                nc.tensor.matmul(
                                    pp[j][:, :cn],
                                    w_all[:, ki * H + 128 * hi : ki * H + 128 * (hi + 1)],
                                    x_all[:, ki * cap + c0 : ki * cap + c0 + cn],
                                    start=(ki == 0),
                                    stop=(ki == KD - 1),
                                )
                        for j, hi in enumerate(his):
                            writer(hi, pp[j])

                def gate_writer(hi, pp):
                    nc.scalar.activation(
                        h_sb[:, cmax * hi : cmax * hi + cn],
                        pp[:, :cn],
                        mybir.ActivationFunctionType.Silu,
                    )

                def up_writer(hi, pp):
                    hslc = slice(cmax * hi, cmax * hi + cn)
                    nc.vector.tensor_mul(h_sb[:, hslc], h_sb[:, hslc], pp[:, :cn])

                phase(wg_s, gate_writer)
                phase(wu_s, up_writer)
                return h_sb

            down_ctr = [0]

            def down(h_sb, c0, cn, wd_sb, last_chunk):
                for di in range(KD):
                    # The very last d-tile runs as two column groups so its
                    # cast+store pipelines against its own matmuls instead of
                    # sitting fully exposed after the final one.
                    if last_chunk and di == KD - 1:
                        cgroups = [(0, cn // 2), (cn // 2, cn)]
                    else:
                        cgroups = [(0, cn)]
                    for g0, g1 in cgroups:
                        gw = g1 - g0
                        po = psum.tile(
                            [128, 512],
                            f32,
                            tag=f"pp{down_ctr[0] % 2}",
                            bufs=1,
                            name="po",
                        )
                        down_ctr[0] += 1
                        for hk in range(KH):
                            nc.tensor.matmul(
                                po[:, :gw],
                                wd_sb[:, hk * D + 128 * di : hk * D + 128 * (di + 1)],
                                h_sb[:, cmax * hk + g0 : cmax * hk + g1],
                                start=(hk == 0),
                                stop=(hk == KH - 1),
                            )
                        o = opool.tile([128, 512], bf16, tag="o", name="o")
                        nc.vector.tensor_copy(o[:, :gw], po[:, :gw])
                        # Store queue time is DRAM-row-rate-bound and these
                        # rows are thin, so split by PARTITION to halve the
                        # rows each queue processes.
                        d0 = 128 * di
                        csl2 = slice(c0 + g0, c0 + g1)
                        nc.sync.dma_start(out[d0 : d0 + 64, csl2], o[0:64, :gw])
                        nc.scalar.dma_start(
                            out[d0 + 64 : d0 + 128, csl2], o[64:128, :gw]
                        )

            # Software-pipelined emission: down(c) goes after gate_up(c+1) so
            # the PE can run chunk c+1's gate matmuls while the DVE finishes
            # chunk c's h tiles (h is double-buffered).
            h0 = gate_up(0, c1, 0)
            h1 = gate_up(c1, c2, 1)
            # Down weights reuse chunk-0 weight slots (same tag -> same SBUF
            # slot; Tile inserts the WAR dependencies) — see NOTE above.
            wd_sb = [
                wpool.tile([128, KH * D], bf16, tag="wu0", name="wd_all0"),
                wpool.tile([128, KH * D], bf16, tag="wg0", name="wd_all1"),
            ]
            nc.gpsimd.dma_start(wd_sb[0][:, 0 : 8 * D], wdT[0][:, 0 : 8 * D])
            nc.gpsimd.dma_start(wd_sb[0][:, 8 * D : 16 * D], wdT[0][:, 8 * D : 16 * D])
            nc.gpsimd.dma_start(wd_sb[1][:, 0 : 8 * D], wdT[1][:, 0 : 8 * D])
            nc.gpsimd.dma_start(wd_sb[1][:, 8 * D : 16 * D], wdT[1][:, 8 * D : 16 * D])
            down(h0, 0, c1, wd_sb[0], False)
            down(h1, c1, c2, wd_sb[1], True)
    _split_multi_waits(nc)
    _NC_CACHE[key] = nc
    return nc


def kernel(x, expert_indices, w_gate, w_up, w_down):
    global LAST_RESULT
    _install_shims()
    from concourse import bass_utils

    x = np.asarray(x)
    ei = np.asarray(expert_indices).astype(np.int64)
    w_gate = np.asarray(w_gate)
    w_up = np.asarray(w_up)
    w_down = np.asarray(w_down)

    flat = ei.reshape(-1)  # pair p = t*A + a  ->  expert id
    # Dedup: a (token, slot) pair whose expert already appears in an earlier
    # slot of the same token produces an identical output row — compute the
    # first occurrence only and copy the result to the duplicates afterward.
    keep = np.ones(T * A, dtype=bool)
    for a in range(1, A):
        dup_any = np.zeros(T, dtype=bool)
        for b in range(a):
            dup_any |= ei[:, a] == ei[:, b]
        keep[a::A] = ~dup_any[:T]
    kept = np.nonzero(keep)[0]
    flat_kept = flat[kept]
    counts = np.bincount(flat_kept, minlength=E)
    order = np.argsort(flat_kept, kind="stable")
    starts = np.zeros(E + 1, dtype=np.int64)
    np.cumsum(counts, out=starts[1:])

    # Two half-loads per core: sort experts by count; the E/2 biggest go to
    # the cores' chunk-0 slots (split in half across core pairs), the E/2
    # smallest to the chunk-1 slots.
    assert E == N_CORES
    by_size = np.argsort(-counts, kind="stable")
    big, small = by_size[: E // 2], by_size[E // 2 :]
    c1 = max(128, int(math.ceil(counts[big[0]] / 2)))
    c2 = max(128, int(math.ceil(counts[small[0]] / 2)))
    assert c1 <= 512 and c2 <= 512, (c1, c2)
    cap = c1 + c2

    KD = D // 128
    KH = H // 128

    def pack(a, k, w):
        # [k*128, w] -> [128, k*w] with [p, ki*w + c] = a[ki*128 + p, c]
        return np.ascontiguousarray(
            a.reshape(k, 128, w).transpose(1, 0, 2).reshape(128, k * w)
        )

    wpk = {}
    for e in range(E):
        wpk[e] = (
            pack(w_gate[e].T.astype(BF16), KD, H),
            pack(w_up[e].T.astype(BF16), KD, H),
            pack(w_down[e].T.astype(BF16), KH, D),
        )

    # slot assignment: core 2i/2i+1 chunk0 <- halves of big[i]; chunk1 <-
    # halves of small[i].
    slot_expert = np.zeros((N_CORES, 2), dtype=np.int64)
    slot_idx = [[None, None] for _ in range(N_CORES)]
    for i in range(E // 2):
        for s, exp_list in ((0, big), (1, small)):
            e = exp_list[i]
            idx = kept[order[starts[e] : starts[e + 1]]]
            half = (len(idx) + 1) // 2
            for j, piece in enumerate((idx[:half], idx[half:])):
                core = 2 * i + j
                slot_expert[core, s] = e
                slot_idx[core][s] = piece

    in_maps = []
    for core in range(N_CORES):
        xeT = np.zeros((D, cap), dtype=BF16)
        for s, coff, cw in ((0, 0, c1), (1, c1, c2)):
            idx = slot_idx[core][s]
            tok = idx // A
            xeT[:, coff : coff + len(idx)] = x[tok].T.astype(BF16)
        e0, e1 = slot_expert[core]
        in_maps.append(
            {
                "xT": pack(xeT, KD, cap),
                "wgT0": wpk[e0][0],
                "wuT0": wpk[e0][1],
                "wdT0": wpk[e0][2],
                "wgT1": wpk[e1][0],
                "wuT1": wpk[e1][1],
                "wdT1": wpk[e1][2],
            }
        )

    nc = _build_nc(c1, c2)
    res = bass_utils.run_bass_kernel_spmd(nc, in_maps, core_ids=list(range(N_CORES)))
    LAST_RESULT = res

    out = np.zeros((T * A, D), dtype=np.float32)
    for core in range(N_CORES):
        oT = np.asarray(res.results[core]["out"])  # [D, cap] bf16
        for s, coff in ((0, 0), (1, c1)):
            idx = slot_idx[core][s]
            out[idx] = oT[:, coff : coff + len(idx)].T.astype(np.float32)
    out = out.reshape(T, A, D)
    for a in range(1, A):  # fill duplicate slots from their first occurrence
        for b in range(a):
            m = ei[:, a] == ei[:, b]
            if b > 0:
                for c in range(b):
                    m &= ei[:, b] != ei[:, c]  # b is itself the first occurrence
            out[m, a] = out[m, b]
    return out
